# revision 49
# baseline (speedup 1.0000x reference)
"""Differentiable K-Means layer (vq_codebook) TRN2 kernel.

Strategy (8 NeuronCores, data-parallel over N = B*S = 32768 rows):
  - shard x row-wise: core i gets x[i] (B=8 -> one batch element per core).
  - host splits x into a bf16 hi/lo pair (exact to ~17 mantissa bits); the
    device reconstitutes f32r (TRN2's full-rate 12-mantissa-bit fp32 matmul
    dtype). The bf16 halves ride the DMA xbar transpose (bf16-only) to build
    x^T tiles for the distance GEMM with zero TensorE transpose cost.
  - host pre-computes c^T and -|c|^2/2 in f32r (pure c-preprocessing).
  - device per core: cross = x @ c^T in f32r at full PE rate, fused
    -|c|^2/2 rank-1 update (PSUM holds the exact logits/(-2/T)), exact
    row-max on DVE, exp on ACT with fused row-sum accumulation, normalize
    on DVE -> assignments out; ws = a^T @ x accumulates in PSUM across all
    32 row tiles and each core outputs its partial ws (K x D).
  - gather step on host: sum the 8 partial ws in fp64, sum_w from the
    gathered assignments, K x K repulsion via GEMMs, and the tiny
    momentum/centroid update (all O(K*D) work on c-sized tensors).
  - emission is software-pipelined (ws MMs for tile n are emitted after the
    cross MMs for tile n+1) so the TensorE static schedule never
    head-of-line blocks on the softmax chain; xbar transposes stay on their
    own HWDGE ring (ACT) to avoid the DMATranspose<->DMACopy xbar HW bug.
Outputs: assignments (B,S,K) sharded back, centroids_new, mom_new.
"""

import numpy as np
import ml_dtypes

B, S, D, K = 8, 4096, 512, 512
N_CORES = 8
NLOC = (B * S) // N_CORES          # 4096 rows per core
NT = NLOC // 128                   # 32 row tiles per core
GROUP = 512                        # rows per transpose group
NGRP = NLOC // GROUP               # 8 groups
TPG = GROUP // 128                 # 4 row tiles per group
DC = D // 128                      # 4 contraction chunks
KC = K // 128                      # 4 centroid chunks

EPS = 1e-7
TEMP = 0.1
MOMENTUM = 0.9
CENTROID_LR = 0.1
REP_STRENGTH = 0.1

_CACHE = {}


def _build_nc():
    import os as _os
    import concourse.bacc as bacc
    import concourse.mybir as mybir
    import concourse.tile as tile

    f32 = mybir.dt.float32
    f32r = mybir.dt.float32r
    bf16 = mybir.dt.bfloat16
    AF = mybir.ActivationFunctionType
    AX = mybir.AxisListType
    OP = mybir.AluOpType

    TLSIM = bool(int(_os.environ.get("KMEANS_TLSIM", "0")))
    DBG = bool(int(_os.environ.get("KMEANS_DBG", "0")))

    nc = bacc.Bacc("TRN2", target_bir_lowering=False, debug=False,
                   num_devices=N_CORES)

    xh_d = nc.dram_tensor("xh", [NLOC, D], bf16, kind="ExternalInput")
    xl_d = nc.dram_tensor("xl", [NLOC, D], bf16, kind="ExternalInput")
    cT_d = nc.dram_tensor("cT", [D, K], f32r, kind="ExternalInput")
    nhcsq_d = nc.dram_tensor("nhcsq", [1, K], f32r, kind="ExternalInput")
    a_d = nc.dram_tensor("a_out", [NLOC, K], f32, kind="ExternalOutput")
    ws_d = nc.dram_tensor("ws_part", [K, D], f32, kind="ExternalOutput")

    with tile.TileContext(nc) as tc:
        with tc.tile_pool(name="const", bufs=1) as const, \
             tc.tile_pool(name="dram", bufs=1, space="DRAM") as dram:
            # ---------------- constants ----------------
            ones_stage = const.tile([128, 128], f32)
            nc.gpsimd.memset(ones_stage[:], 1.0)
            ones_row = const.tile([1, 128], f32r)
            nc.vector.tensor_copy(ones_row[:], ones_stage[0:1, 0:128])

            cT_r = const.tile([128, DC, K], f32r)     # c^T: [d-part, dchunk, k]
            nc.sync.dma_start(cT_r[:], cT_d[:].rearrange("(i p) k -> p i k", p=128))
            nhcsq_row = const.tile([1, K], f32r)      # -|c_k|^2 / 2 as a row
            nc.sync.dma_start(nhcsq_row[:], nhcsq_d[:])

            # ---------------- main loop (pipelined emission) ----------------
            with tc.tile_pool(name="ws_pool", bufs=1, space="PSUM") as wps:
                ws_ps = [wps.tile([128, D], f32, name=f"ws_ps{j}") for j in range(KC)]

                with tc.tile_pool(name="xt", bufs=1) as xtp, \
                     tc.tile_pool(name="work", bufs=3) as wk, \
                     tc.tile_pool(name="cr_ps", bufs=4, space="PSUM") as cps:
                    xhT = {}
                    xlT = {}
                    xrT = {}
                    a_tiles = {}
                    xr_tiles = {}

                    def emit_transposes(g):
                        r0 = g * GROUP
                        xhT[g] = xtp.tile([128, DC, GROUP], bf16,
                                          name=f"xhT{g}", tag="xhT", bufs=NGRP)
                        xlT[g] = xtp.tile([128, DC, GROUP], bf16,
                                          name=f"xlT{g}", tag="xlT", bufs=NGRP)
                        xrT[g] = xtp.tile([128, DC, GROUP], f32r,
                                          name=f"xrT{g}", tag="xrT", bufs=4)
                        for i in range(DC):
                            # xbar transposes must stay on their own HWDGE ring
                            # (ACT): mixing with plain copies on one ring hits
                            # the DMATranspose<->DMACopy xbar-mode HW bug.
                            nc.scalar.dma_start(
                                xhT[g][:, i, :],
                                xh_d[r0:r0 + GROUP, i * 128:(i + 1) * 128],
                                transpose=True)
                            nc.scalar.dma_start(
                                xlT[g][:, i, :],
                                xl_d[r0:r0 + GROUP, i * 128:(i + 1) * 128],
                                transpose=True)
                            nc.vector.tensor_add(xrT[g][:, i, :], xhT[g][:, i, :],
                                                 xlT[g][:, i, :])

                    def emit_cross(n):
                        g, t = n // TPG, n % TPG
                        n0 = n * 128
                        xh_n = wk.tile([128, D], bf16, name=f"xh_n{n}",
                                       tag="xh_n", bufs=4)
                        xl_n = wk.tile([128, D], bf16, name=f"xl_n{n}",
                                       tag="xl_n", bufs=4)
                        nc.sync.dma_start(xh_n[:], xh_d[n0:n0 + 128, :])
                        nc.sync.dma_start(xl_n[:], xl_d[n0:n0 + 128, :])
                        xr_n = wk.tile([128, D], f32r, name=f"xr_n{n}",
                                       tag="xr_n", bufs=6)
                        nc.vector.tensor_add(xr_n[:], xh_n[:], xl_n[:])
                        xr_tiles[n] = xr_n

                        ps_c = cps.tile([128, K], f32, name=f"ps_c{n}", tag="ps_c")
                        for i in range(DC):
                            nc.tensor.matmul(
                                ps_c[:], xrT[g][:, i, t * 128:(t + 1) * 128],
                                cT_r[:, i, :], start=(i == 0), stop=False)
                        nc.tensor.matmul(ps_c[:], ones_row[:], nhcsq_row[:],
                                         start=False, stop=True)

                        mx = wk.tile([128, 1], f32, name=f"mx{n}", tag="mx")
                        nc.vector.tensor_reduce(mx[:], ps_c[:], axis=AX.X, op=OP.max)
                        nbias = wk.tile([128, 1], f32, name=f"nb{n}", tag="nb")
                        nc.vector.tensor_scalar(nbias[:], mx[:], -2.0 / TEMP, None,
                                                op0=OP.mult)
                        a_un = wk.tile([128, K], f32, name=f"a_un{n}", tag="a_un")
                        sume = wk.tile([128, 1], f32, name=f"sume{n}", tag="sume")
                        nc.scalar.activation(a_un[:], ps_c[:], AF.Exp,
                                             bias=nbias[:], scale=2.0 / TEMP,
                                             accum_out=sume[:])
                        den = wk.tile([128, 1], f32, name=f"den{n}", tag="den")
                        nc.vector.tensor_scalar_add(den[:], sume[:], EPS)
                        rcp_n = wk.tile([128, 1], f32, name=f"rcp_n{n}", tag="rcp_n")
                        nc.vector.reciprocal(rcp_n[:], den[:])
                        a_sb = wk.tile([128, K], f32r, name=f"a_sb{n}",
                                       tag="a_sb", bufs=4)
                        nc.vector.tensor_scalar(a_sb[:], a_un[:], rcp_n[:], None,
                                                op0=OP.mult)
                        nc.gpsimd.dma_start(a_d[n0:n0 + 128, :],
                                            a_sb[:].bitcast(mybir.dt.float32))
                        a_tiles[n] = a_sb

                    def emit_ws(n):
                        a_sb = a_tiles.pop(n)
                        xr_n = xr_tiles.pop(n)
                        for j in range(KC):
                            nc.tensor.matmul(
                                ws_ps[j][:], a_sb[:, j * 128:(j + 1) * 128],
                                xr_n[:], start=(n == 0), stop=(n == NT - 1))

                    emit_transposes(0)
                    emit_transposes(1)
                    for n in range(NT + 1):
                        if n < NT:
                            if n % TPG == 0 and (g2 := n // TPG + 2) < NGRP:
                                emit_transposes(g2)
                            emit_cross(n)
                        if n >= 1:
                            emit_ws(n - 1)

                # -------- drain partial weighted sums to the output --------
                ws_sb, _free_ws_sb = tc.tile([128, KC, D], f32, name="ws_sb")
                for j in range(KC):
                    nc.vector.tensor_copy(ws_sb[:, j, :], ws_ps[j][:])
                nc.sync.dma_start(
                    ws_d[:].rearrange("(j p) d -> p j d", p=128), ws_sb[:])
            _free_ws_sb()

    nc.finalize()
    return nc


def _f32r_round(a):
    """Round-to-nearest-even fp32 -> fp32r (1s + 8e + 11 explicit mantissa)."""
    v = np.ascontiguousarray(a, dtype=np.float32).view(np.uint32)
    r = v + 0x7FF + ((v >> 12) & 1)
    r &= np.uint32(0xFFFFF000)
    return r.view(np.float32)


def _prep_inputs(x):
    bf16 = ml_dtypes.bfloat16
    x = np.ascontiguousarray(x, dtype=np.float32).reshape(N_CORES, NLOC, D)
    xh = x.astype(bf16)
    xl = (x - xh.astype(np.float32)).astype(bf16)
    return xh, xl


def _host_repulsion(c64):
    """Pairwise centroid repulsion via GEMMs (no KxKxD tensor)."""
    g = c64 @ c64.T
    csq = np.diag(g).copy()
    sq = np.maximum(csq[:, None] + csq[None, :] - 2.0 * g, 0.0)
    dist = np.sqrt(sq + EPS)
    w = np.maximum(0.0, 1.0 - dist)
    s = REP_STRENGTH * w / (dist + EPS)
    return s.sum(axis=1)[:, None] * c64 - s @ c64


def kernel(x, centroids, centroid_momentum, _trace=False):
    import os
    from concourse import bass_utils

    if "nc" not in _CACHE:
        _CACHE["nc"] = _build_nc()
    nc = _CACHE["nc"]

    c = np.ascontiguousarray(centroids, dtype=np.float32)
    c_r = _f32r_round(c)
    cT = np.ascontiguousarray(c_r.T)
    nhcsq = _f32r_round(
        (-0.5 * (c_r.astype(np.float64) ** 2).sum(axis=1)).astype(np.float32)
    ).reshape(1, K)
    xh, xl = _prep_inputs(x)
    in_maps = []
    for i in range(N_CORES):
        in_maps.append({
            "xh": np.ascontiguousarray(xh[i]),
            "xl": np.ascontiguousarray(xl[i]),
            "cT": cT,
            "nhcsq": nhcsq,
        })

    env_backup = os.environ.get("BASS_NEVER_TRACE")
    if not _trace:
        # the axon NTFF hook is not importable in this container; force the
        # no-trace path even if BASS_TRACE is set globally
        os.environ["BASS_NEVER_TRACE"] = "1"
    try:
        res = bass_utils.run_bass_kernel_spmd(
            nc, in_maps, core_ids=list(range(N_CORES)), trace=_trace)
    finally:
        if not _trace:
            if env_backup is None:
                os.environ.pop("BASS_NEVER_TRACE", None)
            else:
                os.environ["BASS_NEVER_TRACE"] = env_backup
    _CACHE["last_result"] = res

    a = np.stack([res.results[i]["a_out"] for i in range(N_CORES)], axis=0)

    # ---- host-side reduction + centroid/momentum update (gather step) ----
    ws = np.zeros((K, D), dtype=np.float64)
    for i in range(N_CORES):
        ws += res.results[i]["ws_part"].astype(np.float64)
    sum_w = a.reshape(-1, K).sum(axis=0, dtype=np.float64)
    new_c = ws / (sum_w[:, None] + EPS)

    c64 = c.astype(np.float64)
    rep = _host_repulsion(c64)
    u = new_c - c64 + rep
    mn = (MOMENTUM * np.asarray(centroid_momentum, dtype=np.float64)
          + (1.0 - MOMENTUM) * u)
    cn = c64 + CENTROID_LR * mn

    a = a.reshape(B, S, K)
    return a, cn.astype(np.float32), mn.astype(np.float32)


# revision 61
# speedup vs baseline: 1.5967x; 1.5967x over previous
"""Differentiable K-Means layer (vq_codebook) TRN2 kernel.

Strategy (8 NeuronCores, data-parallel over N = B*S = 32768 rows):
  - shard x row-wise: core i gets x[i] (B=8 -> one batch element per core).
  - host splits x into a bf16 hi/lo pair (exact to ~17 mantissa bits); the
    device reconstitutes f32r (TRN2's full-rate 12-mantissa-bit fp32 matmul
    dtype). The bf16 halves ride the DMA xbar transpose (bf16-only) to build
    x^T tiles for the distance GEMM with zero TensorE transpose cost.
  - host pre-computes c^T and -|c|^2/2 in f32r (pure c-preprocessing).
  - device per core: cross = x @ c^T in f32r at full PE rate, fused
    -|c|^2/2 rank-1 update (PSUM holds the exact logits/(-2/T)), exact
    row-max on DVE, exp on ACT with fused row-sum accumulation, normalize
    on DVE -> fp16 assignments out; ws = a^T @ x runs in fp16 (11-bit
    mantissa, full PE rate, half the DMA bytes; PSUM stays fp32) and
    accumulates across all 32 row tiles; each core outputs its partial
    ws (K x D, fp32). The natural-layout x for the ws GEMM is rebuilt
    hi+lo -> fp16 inside the SWDGE datapath (accum_op=add, dtype convert)
    at zero compute-engine cost.
  - gather step on host: sum the 8 partial ws in fp64, sum_w from the
    gathered assignments, K x K repulsion via GEMMs, and the tiny
    momentum/centroid update (all O(K*D) work on c-sized tensors).
  - emission is software-pipelined (ws MMs for tile n are emitted after the
    cross MMs for tile n+1) so the TensorE static schedule never
    head-of-line blocks on the softmax chain. Every sequencer carries one
    stream type: SP = xbar transposes only (mode-pure ring, so the
    DMATranspose<->DMACopy xbar HW bug cannot trigger), Pool/SWDGE = all
    plain copies, ACT = activations only, DVE = softmax math, PE = matmuls.
Outputs: assignments (B,S,K) sharded back, centroids_new, mom_new.
"""

import numpy as np
import ml_dtypes

B, S, D, K = 8, 4096, 512, 512
N_CORES = 8
NLOC = (B * S) // N_CORES          # 4096 rows per core
NT = NLOC // 128                   # 32 row tiles per core
GROUP = 1024                       # rows per transpose group
NGRP = NLOC // GROUP               # 8 groups
TPG = GROUP // 128                 # 4 row tiles per group
DC = D // 128                      # 4 contraction chunks
KC = K // 128                      # 4 centroid chunks

EPS = 1e-7
TEMP = 0.1
MOMENTUM = 0.9
CENTROID_LR = 0.1
REP_STRENGTH = 0.1

_CACHE = {}


def _build_nc():
    import os as _os
    import concourse.bacc as bacc
    import concourse.mybir as mybir
    import concourse.tile as tile

    f32 = mybir.dt.float32
    f32r = mybir.dt.float32r
    f16 = mybir.dt.float16
    bf16 = mybir.dt.bfloat16
    AF = mybir.ActivationFunctionType
    AX = mybir.AxisListType
    OP = mybir.AluOpType

    TLSIM = bool(int(_os.environ.get("KMEANS_TLSIM", "0")))
    DBG = bool(int(_os.environ.get("KMEANS_DBG", "0")))

    nc = bacc.Bacc("TRN2", target_bir_lowering=False, debug=False,
                   num_devices=N_CORES)

    xh_d = nc.dram_tensor("xh", [NLOC, D], bf16, kind="ExternalInput")
    xl_d = nc.dram_tensor("xl", [NLOC, D], bf16, kind="ExternalInput")
    cT_d = nc.dram_tensor("cT", [D, K], f32r, kind="ExternalInput")
    nhcsq_d = nc.dram_tensor("nhcsq", [1, K], f32r, kind="ExternalInput")
    a_d = nc.dram_tensor("a_out", [NLOC, K], f16, kind="ExternalOutput")
    ws_d = nc.dram_tensor("ws_part", [K, D], f32, kind="ExternalOutput")

    with tile.TileContext(nc) as tc:
        with tc.tile_pool(name="const", bufs=1) as const, \
             tc.tile_pool(name="dram", bufs=1, space="DRAM") as dram:
            # ---------------- constants ----------------
            ones_stage = const.tile([128, 128], f32)
            nc.gpsimd.memset(ones_stage[:], 1.0)
            ones_row = const.tile([1, 128], f32r)
            nc.vector.tensor_copy(ones_row[:], ones_stage[0:1, 0:128])

            cT_r = const.tile([128, DC, K], f32r)     # c^T: [d-part, dchunk, k]
            nc.gpsimd.dma_start(cT_r[:], cT_d[:].rearrange("(i p) k -> p i k", p=128))
            nhcsq_row = const.tile([1, K], f32r)      # -|c_k|^2 / 2 as a row
            nc.gpsimd.dma_start(nhcsq_row[:], nhcsq_d[:])

            # ---------------- main loop (pipelined emission) ----------------
            with tc.tile_pool(name="ws_pool", bufs=1, space="PSUM") as wps:
                ws_ps = [wps.tile([128, D], f32, name=f"ws_ps{j}") for j in range(KC)]

                with tc.tile_pool(name="xt", bufs=1) as xtp, \
                     tc.tile_pool(name="work", bufs=3) as wk, \
                     tc.tile_pool(name="cr_ps", bufs=4, space="PSUM") as cps:
                    xhT = {}
                    xlT = {}
                    xrT = {}
                    a_tiles = {}
                    xr_tiles = {}

                    def emit_transposes(g):
                        r0 = g * GROUP
                        xhT[g] = xtp.tile([128, DC, GROUP], bf16,
                                          name=f"xhT{g}", tag="xhT", bufs=3)
                        xlT[g] = xtp.tile([128, DC, GROUP], bf16,
                                          name=f"xlT{g}", tag="xlT", bufs=3)
                        xrT[g] = xtp.tile([128, DC, GROUP], f32r,
                                          name=f"xrT{g}", tag="xrT", bufs=3)
                        for i in range(DC):
                            # xbar transposes get the SP HWDGE ring to
                            # themselves (all plain copies go via SWDGE), so
                            # the DMATranspose<->DMACopy xbar-mode HW bug
                            # cannot trigger and the ACT sequencer only ever
                            # issues activations.
                            nc.sync.dma_start(
                                xhT[g][:, i, :],
                                xh_d[r0:r0 + GROUP, i * 128:(i + 1) * 128],
                                transpose=True)
                            nc.sync.dma_start(
                                xlT[g][:, i, :],
                                xl_d[r0:r0 + GROUP, i * 128:(i + 1) * 128],
                                transpose=True)
                            nc.vector.tensor_add(xrT[g][:, i, :], xhT[g][:, i, :],
                                                 xlT[g][:, i, :])

                    xr_groups = {}

                    def emit_nat_loads(g):
                        r0 = g * GROUP
                        xr_g = wk.tile([128, TPG, D], f16, name=f"xr_g{g}",
                                       tag="xr_g", bufs=2)
                        # SWDGE converts bf16->fp16 on the first copy, then
                        # accumulates the lo half in the DMA datapath -- the
                        # hi/lo reconstitution costs zero compute-engine time
                        nc.gpsimd.dma_start(
                            xr_g[:],
                            xh_d[r0:r0 + GROUP, :].rearrange(
                                "(t p) d -> p t d", p=128))
                        nc.gpsimd.dma_start(
                            xr_g[:],
                            xl_d[r0:r0 + GROUP, :].rearrange(
                                "(t p) d -> p t d", p=128),
                            accum_op=OP.add)
                        xr_groups[g] = xr_g

                    def emit_cross(n):
                        g, t = n // TPG, n % TPG
                        n0 = n * 128
                        xr_tiles[n] = xr_groups[g][:, t, :]

                        ps_c = cps.tile([128, K], f32, name=f"ps_c{n}", tag="ps_c")
                        for i in range(DC):
                            nc.tensor.matmul(
                                ps_c[:], xrT[g][:, i, t * 128:(t + 1) * 128],
                                cT_r[:, i, :], start=(i == 0), stop=False)
                        nc.tensor.matmul(ps_c[:], ones_row[:], nhcsq_row[:],
                                         start=False, stop=True)

                        mx = wk.tile([128, 1], f32, name=f"mx{n}", tag="mx")
                        nc.vector.tensor_reduce(mx[:], ps_c[:], axis=AX.X, op=OP.max)
                        nbias = wk.tile([128, 1], f32, name=f"nb{n}", tag="nb")
                        nc.vector.tensor_scalar(nbias[:], mx[:], -2.0 / TEMP, None,
                                                op0=OP.mult)
                        a_un = wk.tile([128, K], f32, name=f"a_un{n}", tag="a_un")
                        sume = wk.tile([128, 1], f32, name=f"sume{n}", tag="sume")
                        nc.scalar.activation(a_un[:], ps_c[:], AF.Exp,
                                             bias=nbias[:], scale=2.0 / TEMP,
                                             accum_out=sume[:])
                        den = wk.tile([128, 1], f32, name=f"den{n}", tag="den")
                        nc.vector.tensor_scalar_add(den[:], sume[:], EPS)
                        rcp_n = wk.tile([128, 1], f32, name=f"rcp_n{n}", tag="rcp_n")
                        nc.vector.reciprocal(rcp_n[:], den[:])
                        a_sb = wk.tile([128, K], f16, name=f"a_sb{n}",
                                       tag="a_sb", bufs=4)
                        nc.vector.tensor_scalar(a_sb[:], a_un[:], rcp_n[:], None,
                                                op0=OP.mult)
                        nc.gpsimd.dma_start(a_d[n0:n0 + 128, :], a_sb[:])
                        a_tiles[n] = a_sb

                    def emit_ws(n):
                        a_sb = a_tiles.pop(n)
                        xr_n = xr_tiles.pop(n)[:]  # AP slice of the group tile
                        for j in range(KC):
                            nc.tensor.matmul(
                                ws_ps[j][:], a_sb[:, j * 128:(j + 1) * 128],
                                xr_n[:], start=(n == 0), stop=(n == NT - 1))

                    # ---- PE warm-up: dense dummy MMs while the first x
                    # groups stream in, so HAM reaches full clock before the
                    # first real matmul (scratch PSUM, no consumers) ----
                    ones_bf = const.tile([128, 128], bf16, name="ones_bf")
                    nc.vector.tensor_copy(ones_bf[:], ones_stage[:])
                    warm_ps = cps.tile([128, K], f32, name="warm_ps", tag="ps_c")
                    for _w in range(48):
                        nc.tensor.matmul(warm_ps[:, 0:128], ones_bf[:],
                                         ones_bf[:], start=(_w == 0),
                                         stop=(_w == 47))

                    emit_transposes(0)
                    emit_nat_loads(0)
                    emit_transposes(1)
                    emit_nat_loads(1)
                    for n in range(NT + 1):
                        if n < NT:
                            if n % TPG == 0 and (g2 := n // TPG + 2) < NGRP:
                                emit_transposes(g2)
                                emit_nat_loads(g2)
                            emit_cross(n)
                        if n >= 1:
                            emit_ws(n - 1)

                # -------- drain partial weighted sums to the output --------
                ws_sb, _free_ws_sb = tc.tile([128, KC, D], f32, name="ws_sb")
                for j in range(KC):
                    nc.vector.tensor_copy(ws_sb[:, j, :], ws_ps[j][:])
                nc.gpsimd.dma_start(
                    ws_d[:].rearrange("(j p) d -> p j d", p=128), ws_sb[:])
            _free_ws_sb()

    nc.finalize()
    return nc


def _f32r_round(a):
    """Round-to-nearest-even fp32 -> fp32r (1s + 8e + 11 explicit mantissa)."""
    v = np.ascontiguousarray(a, dtype=np.float32).view(np.uint32)
    r = v + 0x7FF + ((v >> 12) & 1)
    r &= np.uint32(0xFFFFF000)
    return r.view(np.float32)


def _prep_inputs(x):
    bf16 = ml_dtypes.bfloat16
    x = np.ascontiguousarray(x, dtype=np.float32).reshape(N_CORES, NLOC, D)
    xh = x.astype(bf16)
    xl = (x - xh.astype(np.float32)).astype(bf16)
    return xh, xl


def _host_repulsion(c64):
    """Pairwise centroid repulsion via GEMMs (no KxKxD tensor)."""
    g = c64 @ c64.T
    csq = np.diag(g).copy()
    sq = np.maximum(csq[:, None] + csq[None, :] - 2.0 * g, 0.0)
    dist = np.sqrt(sq + EPS)
    w = np.maximum(0.0, 1.0 - dist)
    s = REP_STRENGTH * w / (dist + EPS)
    return s.sum(axis=1)[:, None] * c64 - s @ c64


def kernel(x, centroids, centroid_momentum, _trace=False):
    import os
    from concourse import bass_utils

    if "nc" not in _CACHE:
        _CACHE["nc"] = _build_nc()
    nc = _CACHE["nc"]

    c = np.ascontiguousarray(centroids, dtype=np.float32)
    c_r = _f32r_round(c)
    cT = np.ascontiguousarray(c_r.T)
    nhcsq = _f32r_round(
        (-0.5 * (c_r.astype(np.float64) ** 2).sum(axis=1)).astype(np.float32)
    ).reshape(1, K)
    xh, xl = _prep_inputs(x)
    in_maps = []
    for i in range(N_CORES):
        in_maps.append({
            "xh": np.ascontiguousarray(xh[i]),
            "xl": np.ascontiguousarray(xl[i]),
            "cT": cT,
            "nhcsq": nhcsq,
        })

    env_backup = os.environ.get("BASS_NEVER_TRACE")
    if not _trace:
        # the axon NTFF hook is not importable in this container; force the
        # no-trace path even if BASS_TRACE is set globally
        os.environ["BASS_NEVER_TRACE"] = "1"
    try:
        res = bass_utils.run_bass_kernel_spmd(
            nc, in_maps, core_ids=list(range(N_CORES)), trace=_trace)
    finally:
        if not _trace:
            if env_backup is None:
                os.environ.pop("BASS_NEVER_TRACE", None)
            else:
                os.environ["BASS_NEVER_TRACE"] = env_backup
    _CACHE["last_result"] = res

    a = np.stack([res.results[i]["a_out"].astype(np.float32)
                  for i in range(N_CORES)], axis=0)

    # ---- host-side reduction + centroid/momentum update (gather step) ----
    ws = np.zeros((K, D), dtype=np.float64)
    for i in range(N_CORES):
        ws += res.results[i]["ws_part"].astype(np.float64)
    sum_w = a.reshape(-1, K).sum(axis=0, dtype=np.float64)
    new_c = ws / (sum_w[:, None] + EPS)

    c64 = c.astype(np.float64)
    rep = _host_repulsion(c64)
    u = new_c - c64 + rep
    mn = (MOMENTUM * np.asarray(centroid_momentum, dtype=np.float64)
          + (1.0 - MOMENTUM) * u)
    cn = c64 + CENTROID_LR * mn

    a = a.reshape(B, S, K)
    return a, cn.astype(np.float32), mn.astype(np.float32)
